# revision 20
# baseline (speedup 1.0000x reference)
"""Multi-head causal self-attention on 8 Trainium2 NeuronCores.

Problem: x[4, 2048, 768], 12 heads, d_k=64, causal softmax, in/out proj.

Sharding: core c handles batch b = c//2 and head-group hg = c%2
(6 heads = 384 model dims).  Each core computes its 6 heads end-to-end
including the partial output projection; the host sums the two
head-group partials per batch and adds bo.

Per-core layout (all matmuls in float32r, moving dim >= 256 where it
matters for the 1 cycle/row rate):
  - x^T [768, 2048] preloaded (host transposes), weights sliced on host.
  - Q^T, K^T computed as [384, 2048] (3 pair-tiles of 2 heads x 64).
    1/sqrt(d_k) is folded into Wq/bq on the host.
  - V kept untransposed [2048, 384] with a ones column appended per head
    (V' = [V_h | 1]), so the P@V matmul's 65th output row is the softmax
    denominator.
  - Scores computed transposed: S^T[k_block 128, q 512] = K_h @ Q_h^T.
    Head pairs packed on the PE via row groups (contraction dim 64 at
    base partitions 0 / 64 -> tile_position (0,0) and (64,0)).
  - Causal mask: shifted slices of one [128, 896] additive mask tile
    (0 / -1e30) added to the 4 diagonal-supertile blocks before exp.
  - exp on ACT (no max subtraction: scores ~ N(0,1), exp can't overflow).
  - PV: out^T[65, 512] += V'_h^T-chunks @ P^T blocks (K_c = 128).
  - Normalize with DVE reciprocal + gpsimd partition_broadcast, then the
    output projection per 128-row q chunk: out[128, 768] accumulated
    over the 3 pair chunks (K_c = 128).
"""

import numpy as np

D_MODEL = 768
S = 2048
B = 4
DK = 64
NIN = 6  # 768 / 128 input chunks
NPAIR = 3  # 6 local heads as 3 pairs
NS4 = 4  # s chunks of 512
NS = 16  # s chunks of 128
NEG = -1.0e30

_BUILT = {}


def _build_nc():
    if "nc" in _BUILT:
        return _BUILT["nc"]

    import concourse.bass as bass
    import concourse.mybir as mybir
    from concourse import bacc
    from concourse.tile import TileContext

    f32 = mybir.dt.float32
    f32r = mybir.dt.float32r
    Exp = mybir.ActivationFunctionType.Exp

    # Bacc (not plain Bass): its compile() runs move_matmul_waits_to_ldweights
    # and generate_event_semaphores, required to satisfy the 1-wait-per-
    # instruction TRN2 codegen constraint.
    nc = bacc.Bacc()
    xT_d = nc.declare_dram_parameter("xT", [D_MODEL, S], f32r, isOutput=False)
    wq_d = nc.declare_dram_parameter("wq", [D_MODEL, 384], f32r, isOutput=False)
    wk_d = nc.declare_dram_parameter("wk", [D_MODEL, 384], f32r, isOutput=False)
    wv_d = nc.declare_dram_parameter("wv", [D_MODEL, 384], f32r, isOutput=False)
    wo_d = nc.declare_dram_parameter("wo", [384, D_MODEL], f32r, isOutput=False)
    bq_d = nc.declare_dram_parameter("bq", [384, 1], f32, isOutput=False)
    bk_d = nc.declare_dram_parameter("bk", [384, 1], f32, isOutput=False)
    bv_d = nc.declare_dram_parameter("bv", [1, 384], f32, isOutput=False)
    out_d = nc.declare_dram_parameter("out", [S, D_MODEL], f32, isOutput=True)

    with TileContext(nc) as tc:
        with (
            tc.tile_pool(name="big", bufs=6) as big,  # xT chunks then P^T blocks
            tc.tile_pool(name="w", bufs=1) as wpool,
            tc.tile_pool(name="kt", bufs=1) as kpool,
            tc.tile_pool(name="qp", bufs=4) as qpool,
            tc.tile_pool(name="vp", bufs=1) as vpool,
            tc.tile_pool(name="at", bufs=6) as atpool,
            tc.tile_pool(name="ot", bufs=2) as otpool,
            tc.tile_pool(name="mi", bufs=1) as misc,
            tc.tile_pool(name="nm", bufs=2) as norm,
            tc.tile_pool(name="dr", bufs=4, space="DRAM") as drpool,
            tc.tile_pool(name="ps_mm", bufs=3, space="PSUM") as ps_mm,
            tc.tile_pool(name="ps_pv", bufs=2, space="PSUM") as ps_pv,
            tc.tile_pool(name="ps_sm", bufs=2, space="PSUM") as ps_sm,
        ):
            # ---- one-time small loads -------------------------------------
            bq_t = []
            bk_t = []
            for c in range(NPAIR):
                t = misc.tile([128, 1], f32, tag=f"bq{c}")
                nc.sync.dma_start(out=t[:], in_=bq_d[c * 128 : (c + 1) * 128, :])
                bq_t.append(t)
                t = misc.tile([128, 1], f32, tag=f"bk{c}")
                nc.sync.dma_start(out=t[:], in_=bk_d[c * 128 : (c + 1) * 128, :])
                bk_t.append(t)
            bv_b = misc.tile([128, 384], f32, tag="bvb")
            nc.sync.dma_start(
                out=bv_b[:], in_=bv_d[:].partition_broadcast(128).squeeze(1)
            )

            # additive causal mask, sliced with offset 384 - 128*d for the
            # d-th block of a diagonal supertile: keep iff col >= row + 384
            mask_t = misc.tile([128, 896], f32, tag="mask")
            nc.gpsimd.memset(mask_t[:], 0.0)
            nc.gpsimd.affine_select(
                out=mask_t[:],
                in_=mask_t[:],
                compare_op=mybir.AluOpType.is_ge,
                fill=NEG,
                base=-384,
                pattern=[[1, 896]],
                channel_multiplier=-1,
            )

            # ---- weights --------------------------------------------------
            wq_t, wk_t, wv_t = [], [], []
            for c in range(NIN):
                for name, dram, lst in (
                    ("wq", wq_d, wq_t),
                    ("wk", wk_d, wk_t),
                    ("wv", wv_d, wv_t),
                ):
                    t = wpool.tile([128, 384], f32r, tag=f"{name}{c}")
                    nc.sync.dma_start(out=t[:], in_=dram[c * 128 : (c + 1) * 128, :])
                    lst.append(t)
            wo_t = []
            for c in range(NPAIR):
                t = wpool.tile([128, D_MODEL], f32r, tag=f"wo{c}")
                nc.sync.dma_start(out=t[:], in_=wo_d[c * 128 : (c + 1) * 128, :])
                wo_t.append(t)

            # ---- x^T ------------------------------------------------------
            xt = []
            for c in range(NIN):
                t = big.tile([128, S], f32r, tag="big")
                nc.sync.dma_start(out=t[:], in_=xT_d[c * 128 : (c + 1) * 128, :])
                xt.append(t)

            # ---- projections ---------------------------------------------
            # K^T pair tiles [128, 2048] (partition = 2-head dim, free = s)
            kk = [
                kpool.tile([128, S], f32r, tag=f"k{p}", name=f"k{p}")
                for p in range(NPAIR)
            ]
            for p in range(NPAIR):
                for s4 in range(NS4):
                    ps = ps_mm.tile([128, 512], f32, tag="prj", bufs=1)
                    for c in range(NIN):
                        nc.tensor.matmul(
                            ps[:],
                            wk_t[c][:, p * 128 : (p + 1) * 128],
                            xt[c][:, s4 * 512 : (s4 + 1) * 512],
                            start=(c == 0),
                            stop=(c == NIN - 1),
                        )
                    nc.vector.tensor_scalar_add(
                        kk[p][:, s4 * 512 : (s4 + 1) * 512], ps[:], bk_t[p][:]
                    )

            # V' tiles per 128-row s chunk: [128, 6 heads, 65] (col 64 = ones)
            vv = []
            for s in range(NS):
                t = vpool.tile([128, 6, 65], f32r, tag=f"v{s}")
                # whole-tile memset (contiguous); the projection write below
                # covers cols 0:64 of each head, leaving col 64 = 1.0
                # (f32 bitcast: Memset doesn't support the f32r dtype)
                nc.vector.memset(t[:].bitcast(f32), 1.0)
                vv.append(t)
                ps = ps_sm.tile([128, 384], f32, tag="sm")
                for c in range(NIN):
                    nc.tensor.matmul(
                        ps[:],
                        xt[c][:, s * 128 : (s + 1) * 128],
                        wv_t[c][:],
                        start=(c == 0),
                        stop=(c == NIN - 1),
                    )
                nc.vector.tensor_add(
                    t[:, :, 0:64],
                    ps[:].rearrange("p (h d) -> p h d", h=6),
                    bv_b[:].rearrange("p (h d) -> p h d", h=6),
                )

            # Q^T tiles [128, 512] per (pair, s4); s4-major so the rotating
            # qp slots are produced in the order attention consumes them
            q_tiles = {}
            for s4 in range(NS4):
                for p in range(NPAIR):
                    ps = ps_mm.tile([128, 512], f32, tag="prj", bufs=1)
                    for c in range(NIN):
                        nc.tensor.matmul(
                            ps[:],
                            wq_t[c][:, p * 128 : (p + 1) * 128],
                            xt[c][:, s4 * 512 : (s4 + 1) * 512],
                            start=(c == 0),
                            stop=(c == NIN - 1),
                        )
                    qt = qpool.tile([128, 512], f32r, tag="q")
                    nc.vector.tensor_scalar_add(qt[:], ps[:], bq_t[p][:])
                    q_tiles[(p, s4)] = qt

            # ---- attention + output projection, per q supertile ----------
            for Qi in range(NS4):
                attnq = []
                for p in range(NPAIR):
                    qt = q_tiles[(p, Qi)]
                    pv = [
                        ps_pv.tile([65, 512], f32, tag="pv", name="pvA"),
                        ps_pv.tile([65, 512], f32, tag="pv", name="pvB"),
                    ]
                    nkb = Qi * 4 + 4
                    for kb in range(nkb):
                        pss = []
                        for h in range(2):
                            lo, hi = h * 64, h * 64 + 64
                            psc = ps_mm.tile([128, 512], f32, tag="mm")
                            nc.tensor.matmul(
                                psc[:],
                                kk[p][lo:hi, kb * 128 : (kb + 1) * 128],
                                qt[lo:hi, :],
                                start=True,
                                stop=True,
                            )
                            pss.append(psc)
                        if kb >= Qi * 4:
                            off = 384 - (kb - Qi * 4) * 128
                            for h in range(2):
                                nc.vector.tensor_add(
                                    pss[h][:], pss[h][:], mask_t[:, off : off + 512]
                                )
                        for h in range(2):
                            pT = big.tile([128, 512], f32r, tag="pt", bufs=4)
                            nc.scalar.activation(pT[:], pss[h][:], Exp)
                            nc.tensor.matmul(
                                pv[h][:],
                                vv[kb][:, 2 * p + h, :],
                                pT[:],
                                start=(kb == 0),
                                stop=(kb == nkb - 1),
                            )
                    # normalize: row 64 of pv is the softmax denominator
                    at = atpool.tile([128, 512], f32r, tag="attn")
                    attnq.append(at)
                    for h in range(2):
                        rec = norm.tile([1, 512], f32, tag="recip")
                        nc.vector.reciprocal(rec[:], pv[h][64:65, :])
                        # broadcast along partitions via a DRAM bounce
                        # (engines can't read SBUF with zero partition step)
                        dscr = drpool.tile([1, 512], f32, tag="dscr")
                        nc.sync.dma_start(out=dscr[:], in_=rec[:])
                        bc = norm.tile([64, 512], f32, tag="bc")
                        nc.sync.dma_start(
                            out=bc[:], in_=dscr[:].partition_broadcast(64).squeeze(1)
                        )
                        if h == 0:
                            nc.vector.tensor_mul(at[0:64, :], pv[h][0:64, :], bc[:])
                        else:
                            tmp = norm.tile([64, 512], f32r, tag="tmpb")
                            nc.vector.tensor_mul(tmp[:], pv[h][0:64, :], bc[:])
                            # DVE lanes can't shift partitions; DMA moves the
                            # second head's rows into partitions 64..127
                            nc.sync.dma_start(out=at[64:128, :], in_=tmp[:])

                # output projection for this supertile's four 128-row chunks
                for ql in range(4):
                    qc = Qi * 4 + ql
                    po = [
                        ps_sm.tile([128, 384], f32, tag="sm", name="po0"),
                        ps_sm.tile([128, 384], f32, tag="sm", name="po1"),
                    ]
                    for half in range(2):
                        for c in range(NPAIR):
                            nc.tensor.matmul(
                                po[half][:],
                                attnq[c][:, ql * 128 : (ql + 1) * 128],
                                wo_t[c][:, half * 384 : (half + 1) * 384],
                                start=(c == 0),
                                stop=(c == NPAIR - 1),
                            )
                    ot = otpool.tile([128, D_MODEL], f32, tag="out")
                    nc.vector.tensor_copy(ot[:, 0:384], po[0][:])
                    nc.vector.tensor_copy(ot[:, 384:768], po[1][:])
                    nc.sync.dma_start(
                        out=out_d[qc * 128 : (qc + 1) * 128, :], in_=ot[:]
                    )

    if not nc.is_finalized():
        nc.finalize()
    _BUILT["nc"] = nc
    return nc


def _shard_inputs(x, Wq, bq, Wk, bk, Wv, bv, Wo):
    scale = np.float32(1.0 / np.sqrt(DK))
    in_maps = []
    for c in range(8):
        b = c // 2
        hg = c % 2
        cs = slice(hg * 384, hg * 384 + 384)
        in_maps.append(
            {
                "xT": np.ascontiguousarray(x[b].T),
                "wq": np.ascontiguousarray(Wq[:, cs]) * scale,
                "wk": np.ascontiguousarray(Wk[:, cs]),
                "wv": np.ascontiguousarray(Wv[:, cs]),
                "wo": np.ascontiguousarray(Wo[cs, :]),
                "bq": (bq[cs] * scale).reshape(384, 1).astype(np.float32),
                "bk": bk[cs].reshape(384, 1).astype(np.float32),
                "bv": bv[cs].reshape(1, 384).astype(np.float32),
            }
        )
    return in_maps


def kernel(x, Wq, bq, Wk, bk, Wv, bv, Wo, bo, **run_kwargs):
    from concourse.bass_utils import run_bass_kernel_spmd

    arrs = [np.asarray(a, dtype=np.float32) for a in (x, Wq, bq, Wk, bk, Wv, bv, Wo)]
    x, Wq, bq, Wk, bk, Wv, bv, Wo = arrs
    bo = np.asarray(bo, dtype=np.float32)

    nc = _build_nc()
    in_maps = _shard_inputs(x, Wq, bq, Wk, bk, Wv, bv, Wo)
    res = run_bass_kernel_spmd(nc, in_maps, list(range(8)), **run_kwargs)
    out = np.empty((B, S, D_MODEL), np.float32)
    for b in range(B):
        out[b] = res.results[2 * b]["out"] + res.results[2 * b + 1]["out"] + bo
    if run_kwargs:
        kernel.last_results = res
    return out


# revision 24
# speedup vs baseline: 1.4213x; 1.4213x over previous
"""Multi-head causal self-attention on 8 Trainium2 NeuronCores.

Problem: x[4, 2048, 768], 12 heads, d_k=64, causal softmax, in/out proj.

Sharding: core c handles batch b = c//2 and head-group hg = c%2
(6 heads = 384 model dims).  Each core computes its 6 heads end-to-end
including the partial output projection; the host sums the two
head-group partials per batch and adds bo.

Per-core layout (all matmuls in float32r, moving dim >= 256 where it
matters for the 1 cycle/row rate):
  - x^T [768, 2048] preloaded (host transposes), weights sliced on host.
  - Q^T, K^T computed as [384, 2048] (3 pair-tiles of 2 heads x 64).
    1/sqrt(d_k) is folded into Wq/bq on the host.
  - V kept untransposed [2048, 384] with a ones column appended per head
    (V' = [V_h | 1]), so the P@V matmul's 65th output row is the softmax
    denominator.
  - Scores computed transposed: S^T[k_block 128, q 512] = K_h @ Q_h^T.
    Head pairs packed on the PE via row groups (contraction dim 64 at
    base partitions 0 / 64 -> tile_position (0,0) and (64,0)).
  - Causal mask: shifted slices of one [128, 896] additive mask tile
    (0 / -1e30) added to the 4 diagonal-supertile blocks before exp.
  - exp on ACT (no max subtraction: scores ~ N(0,1), exp can't overflow).
  - PV: out^T[65, 512] += V'_h^T-chunks @ P^T blocks (K_c = 128).
  - Normalize with DVE reciprocal + gpsimd partition_broadcast, then the
    output projection per 128-row q chunk: out[128, 768] accumulated
    over the 3 pair chunks (K_c = 128).
"""

import numpy as np

D_MODEL = 768
S = 2048
B = 4
DK = 64
NIN = 6  # 768 / 128 input chunks
NPAIR = 3  # 6 local heads as 3 pairs
NS4 = 4  # s chunks of 512
NS = 16  # s chunks of 128
NEG = -1.0e30

_BUILT = {}


def _build_nc():
    if "nc" in _BUILT:
        return _BUILT["nc"]

    import concourse.bass as bass
    import concourse.mybir as mybir
    from concourse import bacc
    from concourse.tile import TileContext

    f32 = mybir.dt.float32
    f32r = mybir.dt.float32r
    Exp = mybir.ActivationFunctionType.Exp

    # Bacc (not plain Bass): its compile() runs move_matmul_waits_to_ldweights
    # and generate_event_semaphores, required to satisfy the 1-wait-per-
    # instruction TRN2 codegen constraint.
    nc = bacc.Bacc()
    xT_d = nc.declare_dram_parameter("xT", [D_MODEL, S], f32r, isOutput=False)
    wq_d = nc.declare_dram_parameter("wq", [D_MODEL, 384], f32r, isOutput=False)
    wk_d = nc.declare_dram_parameter("wk", [D_MODEL, 384], f32r, isOutput=False)
    wv_d = nc.declare_dram_parameter("wv", [D_MODEL, 384], f32r, isOutput=False)
    wo_d = nc.declare_dram_parameter("wo", [384, D_MODEL], f32r, isOutput=False)
    bq_d = nc.declare_dram_parameter("bq", [384, 1], f32, isOutput=False)
    bk_d = nc.declare_dram_parameter("bk", [384, 1], f32, isOutput=False)
    bv_d = nc.declare_dram_parameter("bv", [1, 384], f32, isOutput=False)
    out_d = nc.declare_dram_parameter("out", [S, D_MODEL], f32, isOutput=True)

    with TileContext(nc) as tc:
        with (
            tc.tile_pool(name="big", bufs=6) as big,  # xT chunks then P^T blocks
            tc.tile_pool(name="w", bufs=1) as wpool,
            tc.tile_pool(name="kt", bufs=1) as kpool,
            tc.tile_pool(name="qp", bufs=4) as qpool,
            tc.tile_pool(name="vp", bufs=1) as vpool,
            tc.tile_pool(name="at", bufs=6) as atpool,
            tc.tile_pool(name="ot", bufs=2) as otpool,
            tc.tile_pool(name="mi", bufs=1) as misc,
            tc.tile_pool(name="nm", bufs=2) as norm,
            tc.tile_pool(name="dr", bufs=4, space="DRAM") as drpool,
            tc.tile_pool(name="ps_mm", bufs=3, space="PSUM") as ps_mm,
            tc.tile_pool(name="ps_pv", bufs=2, space="PSUM") as ps_pv,
            tc.tile_pool(name="ps_sm", bufs=2, space="PSUM") as ps_sm,
        ):
            # ---- one-time small loads -------------------------------------
            bq_t = []
            bk_t = []
            for c in range(NPAIR):
                t = misc.tile([128, 1], f32, tag=f"bq{c}")
                nc.sync.dma_start(out=t[:], in_=bq_d[c * 128 : (c + 1) * 128, :])
                bq_t.append(t)
                t = misc.tile([128, 1], f32, tag=f"bk{c}")
                nc.sync.dma_start(out=t[:], in_=bk_d[c * 128 : (c + 1) * 128, :])
                bk_t.append(t)
            bv_b = misc.tile([128, 384], f32, tag="bvb")
            nc.sync.dma_start(
                out=bv_b[:], in_=bv_d[:].partition_broadcast(128).squeeze(1)
            )

            # additive causal mask, sliced with offset 384 - 128*d for the
            # d-th block of a diagonal supertile: keep iff col >= row + 384
            mask_t = misc.tile([128, 896], f32, tag="mask")
            nc.gpsimd.memset(mask_t[:], 0.0)
            nc.gpsimd.affine_select(
                out=mask_t[:],
                in_=mask_t[:],
                compare_op=mybir.AluOpType.is_ge,
                fill=NEG,
                base=-384,
                pattern=[[1, 896]],
                channel_multiplier=-1,
            )

            # ---- weights --------------------------------------------------
            wq_t, wk_t, wv_t = [], [], []
            for c in range(NIN):
                for name, dram, lst in (
                    ("wq", wq_d, wq_t),
                    ("wk", wk_d, wk_t),
                    ("wv", wv_d, wv_t),
                ):
                    t = wpool.tile([128, 384], f32r, tag=f"{name}{c}")
                    nc.sync.dma_start(out=t[:], in_=dram[c * 128 : (c + 1) * 128, :])
                    lst.append(t)
            wo_t = []
            for c in range(NPAIR):
                t = wpool.tile([128, D_MODEL], f32r, tag=f"wo{c}")
                nc.sync.dma_start(out=t[:], in_=wo_d[c * 128 : (c + 1) * 128, :])
                wo_t.append(t)

            # ---- x^T ------------------------------------------------------
            xt = []
            for c in range(NIN):
                t = big.tile([128, S], f32r, tag="big")
                nc.sync.dma_start(out=t[:], in_=xT_d[c * 128 : (c + 1) * 128, :])
                xt.append(t)

            # ---- projections ---------------------------------------------
            # K^T pair tiles [128, 2048] (partition = 2-head dim, free = s)
            kk = [
                kpool.tile([128, S], f32r, tag=f"k{p}", name=f"k{p}")
                for p in range(NPAIR)
            ]
            for p in range(NPAIR):
                for s4 in range(NS4):
                    ps = ps_sm.tile([128, 512], f32, tag="sm", name="psk")
                    for c in range(NIN):
                        nc.tensor.matmul(
                            ps[:],
                            wk_t[c][:, p * 128 : (p + 1) * 128],
                            xt[c][:, s4 * 512 : (s4 + 1) * 512],
                            start=(c == 0),
                            stop=(c == NIN - 1),
                        )
                    nc.vector.tensor_scalar_add(
                        kk[p][:, s4 * 512 : (s4 + 1) * 512], ps[:], bk_t[p][:]
                    )

            # V' tiles per 128-row s chunk: [128, 6 heads, 65] (col 64 = ones)
            vv = []
            for s in range(NS):
                t = vpool.tile([128, 6, 65], f32r, tag=f"v{s}")
                # whole-tile memset (contiguous); the projection write below
                # covers cols 0:64 of each head, leaving col 64 = 1.0
                # (f32 bitcast: Memset doesn't support the f32r dtype)
                nc.vector.memset(t[:].bitcast(f32), 1.0)
                vv.append(t)
                ps = ps_sm.tile([128, 384], f32, tag="sm")
                for c in range(NIN):
                    nc.tensor.matmul(
                        ps[:],
                        xt[c][:, s * 128 : (s + 1) * 128],
                        wv_t[c][:],
                        start=(c == 0),
                        stop=(c == NIN - 1),
                    )
                nc.vector.tensor_add(
                    t[:, :, 0:64],
                    ps[:].rearrange("p (h d) -> p h d", h=6),
                    bv_b[:].rearrange("p (h d) -> p h d", h=6),
                )

            # Q^T tiles [128, 512] per (pair, s4); s4-major so the rotating
            # qp slots are produced in the order attention consumes them
            q_tiles = {}
            for s4 in range(NS4):
                for p in range(NPAIR):
                    ps = ps_sm.tile([128, 512], f32, tag="sm", name="psq")
                    for c in range(NIN):
                        nc.tensor.matmul(
                            ps[:],
                            wq_t[c][:, p * 128 : (p + 1) * 128],
                            xt[c][:, s4 * 512 : (s4 + 1) * 512],
                            start=(c == 0),
                            stop=(c == NIN - 1),
                        )
                    qt = qpool.tile([128, 512], f32r, tag="q")
                    nc.vector.tensor_scalar_add(qt[:], ps[:], bq_t[p][:])
                    q_tiles[(p, s4)] = qt

            # ---- attention + output projection, per q supertile ----------
            for Qi in range(NS4):
                attnq = []
                for p in range(NPAIR):
                    qt = q_tiles[(p, Qi)]
                    pv = [
                        ps_pv.tile([65, 512], f32, tag="pv", name="pvA", bufs=3),
                        ps_pv.tile([65, 512], f32, tag="pv", name="pvB", bufs=3),
                    ]
                    nkb = Qi * 4 + 4
                    for kb in range(nkb):
                        # causality: diagonal-supertile block d sees only
                        # q_local >= d*128.  Compute/exp/PV only that window
                        # (widened to 256 for the f32r full-rate minimum).
                        d = kb - Qi * 4
                        if d < 0:
                            o_mm, o_t = 0, 0  # fully visible block
                        else:
                            o_t = d * 128
                            o_mm = min(o_t, 256)
                        pss = []
                        for h in range(2):
                            lo, hi = h * 64, h * 64 + 64
                            psc = ps_mm.tile([128, 512], f32, tag="mm")
                            nc.tensor.matmul(
                                psc[:, o_mm:512],
                                kk[p][lo:hi, kb * 128 : (kb + 1) * 128],
                                qt[lo:hi, o_mm:512],
                                start=True,
                                stop=True,
                            )
                            pss.append(psc)
                        if d >= 0:
                            # triangular mask on the [128,128] diagonal block
                            for h in range(2):
                                nc.vector.tensor_add(
                                    pss[h][:, o_t : o_t + 128],
                                    pss[h][:, o_t : o_t + 128],
                                    mask_t[:, 384:512],
                                )
                        for h in range(2):
                            pT = big.tile([128, 512], f32r, tag="pt", bufs=6)
                            nc.scalar.activation(
                                pT[:, o_t:512], pss[h][:, o_t:512], Exp
                            )
                            if o_t > o_mm:
                                # d == 3: PV window is wider than the exp
                                # window; zero the fully-masked gap
                                nc.vector.memset(
                                    pT[:, o_mm:o_t].bitcast(f32), 0.0
                                )
                            nc.tensor.matmul(
                                pv[h][:, o_mm:512],
                                vv[kb][:, 2 * p + h, :],
                                pT[:, o_mm:512],
                                start=(kb == 0),
                                stop=(kb == nkb - 1),
                            )
                    # normalize: row 64 of pv is the softmax denominator
                    at = atpool.tile([128, 512], f32r, tag="attn")
                    attnq.append(at)
                    for h in range(2):
                        rec = norm.tile([1, 512], f32, tag="recip")
                        nc.vector.reciprocal(rec[:], pv[h][64:65, :])
                        # broadcast along partitions via a DRAM bounce
                        # (engines can't read SBUF with zero partition step)
                        dscr = drpool.tile([1, 512], f32, tag="dscr")
                        nc.sync.dma_start(out=dscr[:], in_=rec[:])
                        bc = norm.tile([64, 512], f32, tag="bc")
                        nc.sync.dma_start(
                            out=bc[:], in_=dscr[:].partition_broadcast(64).squeeze(1)
                        )
                        if h == 0:
                            nc.vector.tensor_mul(at[0:64, :], pv[h][0:64, :], bc[:])
                        else:
                            tmp = norm.tile([64, 512], f32r, tag="tmpb")
                            nc.vector.tensor_mul(tmp[:], pv[h][0:64, :], bc[:])
                            # DVE lanes can't shift partitions; DMA moves the
                            # second head's rows into partitions 64..127
                            nc.sync.dma_start(out=at[64:128, :], in_=tmp[:])

                # output projection for this supertile's four 128-row chunks
                for ql in range(4):
                    qc = Qi * 4 + ql
                    po = [
                        ps_sm.tile([128, 384], f32, tag="sm", name="po0"),
                        ps_sm.tile([128, 384], f32, tag="sm", name="po1"),
                    ]
                    for half in range(2):
                        for c in range(NPAIR):
                            nc.tensor.matmul(
                                po[half][:],
                                attnq[c][:, ql * 128 : (ql + 1) * 128],
                                wo_t[c][:, half * 384 : (half + 1) * 384],
                                start=(c == 0),
                                stop=(c == NPAIR - 1),
                            )
                    ot = otpool.tile([128, D_MODEL], f32, tag="out")
                    nc.vector.tensor_copy(ot[:, 0:384], po[0][:])
                    nc.vector.tensor_copy(ot[:, 384:768], po[1][:])
                    nc.sync.dma_start(
                        out=out_d[qc * 128 : (qc + 1) * 128, :], in_=ot[:]
                    )

    if not nc.is_finalized():
        nc.finalize()
    _BUILT["nc"] = nc
    return nc


def _shard_inputs(x, Wq, bq, Wk, bk, Wv, bv, Wo):
    scale = np.float32(1.0 / np.sqrt(DK))
    in_maps = []
    for c in range(8):
        b = c // 2
        hg = c % 2
        cs = slice(hg * 384, hg * 384 + 384)
        in_maps.append(
            {
                "xT": np.ascontiguousarray(x[b].T),
                "wq": np.ascontiguousarray(Wq[:, cs]) * scale,
                "wk": np.ascontiguousarray(Wk[:, cs]),
                "wv": np.ascontiguousarray(Wv[:, cs]),
                "wo": np.ascontiguousarray(Wo[cs, :]),
                "bq": (bq[cs] * scale).reshape(384, 1).astype(np.float32),
                "bk": bk[cs].reshape(384, 1).astype(np.float32),
                "bv": bv[cs].reshape(1, 384).astype(np.float32),
            }
        )
    return in_maps


def kernel(x, Wq, bq, Wk, bk, Wv, bv, Wo, bo, **run_kwargs):
    from concourse.bass_utils import run_bass_kernel_spmd

    arrs = [np.asarray(a, dtype=np.float32) for a in (x, Wq, bq, Wk, bk, Wv, bv, Wo)]
    x, Wq, bq, Wk, bk, Wv, bv, Wo = arrs
    bo = np.asarray(bo, dtype=np.float32)

    nc = _build_nc()
    in_maps = _shard_inputs(x, Wq, bq, Wk, bk, Wv, bv, Wo)
    res = run_bass_kernel_spmd(nc, in_maps, list(range(8)), **run_kwargs)
    out = np.empty((B, S, D_MODEL), np.float32)
    for b in range(B):
        out[b] = res.results[2 * b]["out"] + res.results[2 * b + 1]["out"] + bo
    if run_kwargs:
        kernel.last_results = res
    return out
